# revision 17
# baseline (speedup 1.0000x reference)
# DIFFormerConv (linear attention) Trainium2 kernel — 8-core data-parallel.
#
# Math (per head h, D=64, H=4, N nodes):
#   q = x@Wq^T + bq ; k = x@Wk^T + bk ; v = x@Wv^T + bv
#   qn = q/|q|, kn = k/|k|  (row-wise per head)
#   kvs = sum_l kn_l (x) v_l   [H,D,D];  vs = sum_l v_l;  ks = sum_l kn_l
#   out = mean_h (qn@kvs + vs) / (qn.ks + N)
#
# Key structure (stage 2 — transpose-free):
#   * q is computed TRANSPOSED directly on the PE: qT = Wq @ x^T with
#     lhsT = wqT column blocks and rhs = the (already transposed) xT
#     input. Two heads stack per PSUM tile ([128, nodes] = h0 rows 0:64,
#     h1 rows 64:128). This removes every DMA-transpose the previous
#     design needed, at identical PE stream cost.
#   * r = |q| per head rides as a separate [4, nodes] row tile, computed
#     by PE block-ones matmuls over q^2 (partition-dim reduction).
#   * q-normalization cancels in the ratio:
#       (qn@kvs+vs)/(qn.ks+N) == (q@kvs + r*vs)/(q.ks + r*N)
#   * per-node denominators den = q.(4ks) + r.4N come from 3 small PE
#     matmuls in transposed space; 1/den is broadcast back to 128
#     partitions with a selector matmul, and q is scaled by it with
#     bf16 DVE muls. The final num matmul contracts head PAIRS at once
#     (lhsT = stacked kvs), plus one [4,w] matmul for the r*vs term.
#   * v-bias deferred: kvs += ks (x) bv, vs += N*bv (post-AllReduce).
#   * the AllReduce payload is packed to [33,512] f32 (68KB).
#   * output is written transposed ([64, NL]) and flipped on the host.

import numpy as np

N_FULL = 100000
IN = 256
D = 64
H = 4
HD = 256
NCORES = 8
NLOC = N_FULL // NCORES          # 12500
NT = 98                          # l-tiles of 128 (padded)
NL = NT * 128                    # 12544
NG = NT // 2                     # 49 groups of 256 nodes
PAD0 = NLOC - (NT - 1) * 128     # used rows in last tile = 84
NCH = 25                         # chunks of 512 nodes (last = 256)

_nc_cache = None
_last_result = None


def _build_nc():
    from contextlib import ExitStack

    import concourse.bass as bass
    import concourse.mybir as mybir
    import concourse.tile as tile
    from concourse import bacc
    from concourse.bass import ds
    from concourse.masks import make_identity
    from bass_rust import add_dep_helper

    f32 = mybir.dt.float32
    bf16 = mybir.dt.bfloat16

    nc = bacc.Bacc()

    xT = nc.dram_tensor("xT", [IN, NL], bf16, kind="ExternalInput")
    wqT = nc.dram_tensor("wqT", [IN, HD], f32, kind="ExternalInput")
    wkT = nc.dram_tensor("wkT", [IN, HD], f32, kind="ExternalInput")
    wvT = nc.dram_tensor("wvT", [IN, HD], f32, kind="ExternalInput")
    bq = nc.dram_tensor("bq", [1, HD], f32, kind="ExternalInput")
    bk = nc.dram_tensor("bk", [1, HD], f32, kind="ExternalInput")
    bv = nc.dram_tensor("bv", [1, HD], f32, kind="ExternalInput")
    padmask = nc.dram_tensor("padmask", [128, 1], f32, kind="ExternalInput")
    outT = nc.dram_tensor("outT", [D, NL], f32, kind="ExternalOutput")

    with tile.TileContext(nc) as tc, ExitStack() as ctx:
        consts = ctx.enter_context(tc.tile_pool(name="consts", bufs=1))
        xtpool = ctx.enter_context(tc.tile_pool(name="xtpool", bufs=3))
        knvpool = ctx.enter_context(tc.tile_pool(name="knvpool", bufs=3))
        spool = ctx.enter_context(tc.tile_pool(name="spool", bufs=3))
        qapool = ctx.enter_context(tc.tile_pool(name="qapool", bufs=1))
        post = ctx.enter_context(tc.tile_pool(name="post", bufs=1))
        dpool = ctx.enter_context(tc.tile_pool(name="dpool", bufs=3))
        opool = ctx.enter_context(tc.tile_pool(name="opool", bufs=3))

        ps_kv = ctx.enter_context(tc.tile_pool(name="ps_kv", bufs=1, space="PSUM"))
        ps_q01 = ctx.enter_context(tc.tile_pool(name="ps_q01", bufs=1, space="PSUM"))
        ps_q23 = ctx.enter_context(tc.tile_pool(name="ps_q23", bufs=1, space="PSUM"))
        ps_r2 = ctx.enter_context(tc.tile_pool(name="ps_r2", bufs=1, space="PSUM"))
        ps_acc = ctx.enter_context(tc.tile_pool(name="ps_acc", bufs=1, space="PSUM"))
        dram = ctx.enter_context(tc.tile_pool(name="dram", bufs=1, space="DRAM"))

        # ---- constants -------------------------------------------------
        # wq kept separate (used as [128,128] head-pair column blocks);
        # wk|wv merged so one N=512 matmul produces k and v together.
        wq_sb = consts.tile([128, 2, HD], bf16, tag="wq_sb")
        nc.gpsimd.dma_start(
            out=wq_sb, in_=wqT[:, :].rearrange("(cb p) f -> p cb f", p=128)
        )
        wkv_sb = consts.tile([128, 2, 512], bf16, tag="wkv_sb")
        nc.gpsimd.dma_start(
            out=wkv_sb[:, :, 0:HD],
            in_=wkT[:, :].rearrange("(cb p) f -> p cb f", p=128),
        )
        nc.gpsimd.dma_start(
            out=wkv_sb[:, :, ds(HD, HD)],
            in_=wvT[:, :].rearrange("(cb p) f -> p cb f", p=128),
        )

        bq_sb = consts.tile([1, HD], bf16, tag="bq")
        nc.gpsimd.dma_start(out=bq_sb, in_=bq[:, :])
        # k bias padded with zeros on the v half (v-bias deferred)
        bkv_sb = consts.tile([1, 512], bf16, tag="bkv")
        nc.vector.memset(bkv_sb, 0.0)
        nc.gpsimd.dma_start(out=bkv_sb[:, 0:HD], in_=bk[:, :])

        # bv as [4, 64] rows (per head) and stacked pair columns
        bv4 = consts.tile([4, D], f32, tag="bv4")
        nc.gpsimd.dma_start(out=bv4, in_=bv[:, :])
        bv01_bc = consts.tile([128, D], f32, tag="bv01_bc")
        bv23_bc = consts.tile([128, D], f32, tag="bv23_bc")
        for j, tgt in ((0, bv01_bc), (2, bv23_bc)):
            for i in range(2):
                h = j + i
                src = bv[:, ds(h * 64, 64)]
                nc.gpsimd.dma_start(
                    out=tgt[ds(i * 64, 64), :],
                    in_=bass.AP(
                        tensor=src.tensor, offset=src.offset,
                        ap=[[0, 64]] + src.ap[1:],
                    ),
                )
        padmask_sb = consts.tile([128, 1], f32, tag="padmask_sb")
        nc.sync.dma_start(out=padmask_sb, in_=padmask[:, :])

        ones_row = consts.tile([1, 512], bf16, tag="ones_row")
        nc.vector.memset(ones_row, 1.0)
        ones_col = consts.tile([128, 1], bf16, tag="ones_col")
        nc.vector.memset(ones_col, 1.0)
        ident = consts.tile([64, 64], f32, tag="ident")
        make_identity(nc, ident)

        # block-ones selectors for partition-dim head reductions
        # boA: cols 0,1 = ones on partitions 0:64 / 64:128 (for sq01)
        # boB: cols 2,3 = ones on partitions 0:64 / 64:128 (for sq23)
        boA = consts.tile([128, 4], bf16, tag="boA")
        boB = consts.tile([128, 4], bf16, tag="boB")
        nc.vector.memset(boA, 0.0)
        nc.vector.memset(boB, 0.0)
        nc.vector.memset(boA[0:64, 0:1], 1.0)
        nc.vector.memset(boA[64:128, 1:2], 1.0)
        nc.vector.memset(boB[0:64, 2:3], 1.0)
        nc.vector.memset(boB[64:128, 3:4], 1.0)
        # selectors broadcasting rec rows back to head-pair partitions
        # (host-provided: DVE memset can't target partition bases 1..3)
        selc = nc.dram_tensor("selc", [8, 128], bf16, kind="ExternalInput")
        sel01 = consts.tile([4, 128], bf16, tag="sel01")
        sel23 = consts.tile([4, 128], bf16, tag="sel23")
        nc.gpsimd.dma_start(out=sel01, in_=selc[0:4, :])
        nc.gpsimd.dma_start(out=sel23, in_=selc[4:8, :])

        # persistent transposed-q store + per-head norms
        qT_all = qapool.tile([128, 2, NCH, 512], bf16, tag="qT_all")
        rT_all = qapool.tile([4, NCH, 512], bf16, tag="rT_all")

        # kvs accumulators: kn01^T @ [v | 1] and kn23^T @ [v | 1]
        kv01_acc = ps_acc.tile([128, 257], f32, tag="kv01_acc")
        kv23_acc = ps_acc.tile([128, 257], f32, tag="kv23_acc")
        vs_acc = ps_acc.tile([1, 257], f32, tag="vs_acc")

        def chunk_width(ci):
            return 512 if ci < NCH - 1 else 256

        # ---- main loop: chunks of 512 nodes (2 groups of 256) ---------
        for ci in range(NCH):
            w = chunk_width(ci)
            ngr = w // 256
            xt_g = xtpool.tile([128, 2, 512], bf16, tag="xt_g", name=f"xt_{ci}")
            nc.sync.dma_start(
                out=xt_g[:, :, 0:w],
                in_=xT[:, ds(ci * 512, w)].rearrange("(cb c) l -> c cb l", c=128),
            )

            # --- qT: two head-pair stacks, bias via ones_row ---
            qT01_ps = ps_q01.tile([128, 512], f32, tag="qT01", name=f"qT01_{ci}")
            qT23_ps = ps_q23.tile([128, 512], f32, tag="qT23", name=f"qT23_{ci}")
            for pair, q_ps in ((0, qT01_ps), (1, qT23_ps)):
                for cb in range(2):
                    nc.tensor.matmul(
                        q_ps[:, 0:w],
                        wq_sb[:, cb, ds(pair * 128, 128)],
                        xt_g[:, cb, 0:w],
                        start=(cb == 0),
                        stop=False,
                    )
                nc.tensor.matmul(
                    q_ps[:, 0:w],
                    bq_sb[:, ds(pair * 128, 128)],
                    ones_row[:, 0:w],
                    start=False,
                    stop=True,
                )
            qc01 = qT_all[:, 0, ci, :]
            qc23 = qT_all[:, 1, ci, :]
            nc.scalar.copy(out=qc01[:, 0:w], in_=qT01_ps[:, 0:w])
            nc.vector.tensor_copy(qc23[:, 0:w], qT23_ps[:, 0:w])

            # --- r2 = per-head column sums of q^2 (PE block-ones) ---
            sq01 = spool.tile([128, 512], bf16, tag="sq01", name=f"sq01_{ci}")
            sq23 = spool.tile([128, 512], bf16, tag="sq23", name=f"sq23_{ci}")
            nc.vector.tensor_mul(sq01[:, 0:w], qc01[:, 0:w], qc01[:, 0:w])
            nc.vector.tensor_mul(sq23[:, 0:w], qc23[:, 0:w], qc23[:, 0:w])
            r2_ps = ps_r2.tile([4, 512], f32, tag="r2", name=f"r2_{ci}")
            nc.tensor.matmul(
                r2_ps[:, 0:w], boA, sq01[:, 0:w], start=True, stop=False
            )
            nc.tensor.matmul(
                r2_ps[:, 0:w], boB, sq23[:, 0:w], start=False, stop=True
            )
            nc.scalar.sqrt(rT_all[:, ci, 0:w], r2_ps[:, 0:w])

            # --- k,v (normal layout) + kn + kvs, per 256-node group ---
            for gi in range(ngr):
                g = 2 * ci + gi
                kv_ps = ps_kv.tile(
                    [128, 2, 512], f32, tag="kv_ps", name=f"kv_ps_{g}"
                )
                for t in range(2):
                    lhs0 = xt_g[:, 0, ds(gi * 256 + t * 128, 128)]
                    lhs1 = xt_g[:, 1, ds(gi * 256 + t * 128, 128)]
                    nc.tensor.matmul(
                        kv_ps[:, t, :], lhs0, wkv_sb[:, 0, :], start=True, stop=False
                    )
                    nc.tensor.matmul(
                        kv_ps[:, t, :], lhs1, wkv_sb[:, 1, :], start=False, stop=False
                    )
                    nc.tensor.matmul(
                        kv_ps[:, t, :], ones_row[:, 0:128], bkv_sb,
                        start=False, stop=True,
                    )

                k_ps4 = kv_ps[:, :, 0:HD].rearrange("p t (h d) -> p t h d", h=H)
                kbf = spool.tile([128, 2, HD], bf16, tag="kbf", name=f"kbf_{g}")
                nc.vector.tensor_copy(kbf, kv_ps[:, :, 0:HD])
                kbf4 = kbf.rearrange("p t (h d) -> p t h d", h=H)
                sqk = spool.tile([128, 2, H, 64], bf16, tag="sqk", name=f"sqk_{g}")
                nc.gpsimd.tensor_mul(sqk, kbf4, kbf4)
                rk2 = spool.tile([128, 2, H, 1], f32, tag="rk2", name=f"rk2_{g}")
                nc.vector.tensor_reduce(
                    rk2, sqk, axis=mybir.AxisListType.X, op=mybir.AluOpType.add
                )
                rk = spool.tile([128, 2, H, 1], f32, tag="rk", name=f"rk_{g}")
                nc.scalar.sqrt(rk, rk2)
                wk_s = spool.tile([128, 2, H, 1], f32, tag="wk_s", name=f"wk_s_{g}")
                nc.vector.reciprocal(wk_s, rk)

                knv = knvpool.tile([128, 2, 513], bf16, tag="knv", name=f"knv_{g}")
                knv4 = knv[:, :, 0:HD].rearrange("p t (h d) -> p t h d", h=H)
                nc.gpsimd.tensor_mul(
                    knv4, kbf4, wk_s[:, :, :, :].to_broadcast([128, 2, H, 64])
                )
                nc.vector.memset(knv[:, :, 512:513], 1.0)
                if g == NG - 1:
                    nc.vector.tensor_scalar_mul(
                        knv[:, 1, 0:HD], knv[:, 1, 0:HD], padmask_sb
                    )
                nc.scalar.copy(out=knv[:, :, ds(HD, HD)], in_=kv_ps[:, :, ds(HD, HD)])

                first = g == 0
                last = g == NG - 1
                for t in range(2):
                    st = first and t == 0
                    sp = last and t == 1
                    nc.tensor.matmul(
                        kv01_acc, knv[:, t, 0:128], knv[:, t, ds(HD, 257)],
                        start=st, stop=sp,
                    )
                    nc.tensor.matmul(
                        kv23_acc, knv[:, t, ds(128, 128)], knv[:, t, ds(HD, 257)],
                        start=st, stop=sp,
                    )
                    nc.tensor.matmul(
                        vs_acc, ones_col, knv[:, t, ds(HD, 257)],
                        start=st, stop=sp,
                    )

        # ---- pack + AllReduce ([33,512] f32 = 68KB) -------------------
        # rows 0:32 = kvs blocks (as [64,256]: col-block h = kvs_h),
        # row 32 = [ks 256 | vs 256]
        kv01_sb = post.tile([128, 257], f32, tag="kv01_sb")
        kv23_sb = post.tile([128, 257], f32, tag="kv23_sb")
        vs_sb = post.tile([1, 257], f32, tag="vs_sb")
        nc.vector.tensor_copy(kv01_sb, kv01_acc)
        nc.vector.tensor_copy(kv23_sb, kv23_acc)
        nc.vector.tensor_copy(vs_sb, vs_acc)

        ar_in = dram.tile([33, 512], f32, tag="ar_in")
        ar_out = dram.tile([33, 512], f32, tag="ar_out")
        arv = ar_in[0:32, :].rearrange("p (a c) -> (p a) c", a=2)  # [64, 256]
        blocks = [
            (kv01_sb, 0, 0), (kv01_sb, 64, 64),
            (kv23_sb, 0, 128), (kv23_sb, 64, 192),
        ]
        ar_dmas = []
        for h, (src, r0, c0) in enumerate(blocks):
            ar_dmas.append(
                nc.sync.dma_start(
                    out=arv[:, ds(h * 64, 64)], in_=src[ds(r0, 64), ds(c0, 64)]
                )
            )
            ar_dmas.append(
                nc.scalar.dma_start(
                    out=ar_in[32:33, ds(h * 64, 64)], in_=src[ds(r0, 64), 256:257]
                )
            )
        ar_dmas.append(
            nc.sync.dma_start(out=ar_in[32:33, ds(HD, HD)], in_=vs_sb[:, 0:HD])
        )
        pnop = nc.gpsimd.nop()
        for dd in ar_dmas:
            add_dep_helper(pnop.ins, dd.ins, sync=True, reason="absorb ar_in dma")
        nc.gpsimd.collective_compute(
            "AllReduce",
            mybir.AluOpType.add,
            ins=[ar_in[:, :].opt()],
            outs=[ar_out[:, :].opt()],
            replica_groups=[list(range(NCORES))],
        )

        # ---- post-reduce fixups ---------------------------------------
        # sums row
        sumr = post.tile([1, 512], f32, tag="sumr")
        nc.sync.dma_start(out=sumr, in_=ar_out[32:33, :])
        arov = ar_out[0:32, :].rearrange("p (a c) -> (p a) c", a=2)  # [64, 256]

        # stacked kvs lhsT tiles [128, 64]: rows 0:64 = kvs_h(even),
        # 64:128 = kvs_h(odd); then += ks (x) bv and cast to bf16
        kvstk = []
        for j in range(2):  # pair 01, 23
            stk = post.tile([128, D], f32, tag=f"kvstk{j}")
            nc.sync.dma_start(out=stk[0:64, :], in_=arov[:, ds((2 * j) * 64, 64)])
            nc.sync.dma_start(
                out=stk[64:128, :], in_=arov[:, ds((2 * j + 1) * 64, 64)]
            )
            kvstk.append(stk)
        # ks columns stacked to match: [128,1] per pair
        ksc = post.tile([128, 2, 1], f32, tag="ksc")
        for j in range(2):
            for i in range(2):
                h = 2 * j + i
                nc.sync.dma_start(
                    out=ksc[ds(i * 64, 64), j, :],
                    in_=ar_out[32:33, ds(h * 64, 64)],
                )
        tmpo = post.tile([128, D], f32, tag="tmpo")
        lhs_bf = []
        for j, bvbc in ((0, bv01_bc), (1, bv23_bc)):
            nc.vector.tensor_scalar_mul(tmpo, bvbc, ksc[:, j, :])
            lb = post.tile([128, D], bf16, tag=f"lhsbf{j}")
            nc.vector.tensor_add(lb, kvstk[j], tmpo)
            lhs_bf.append(lb)
        # vs rows [4, 64] + N*bv, bf16
        vs4 = post.tile([4, D], f32, tag="vs4")
        nc.sync.dma_start(out=vs4, in_=ar_out[32:33, ds(HD, HD)])
        vsM = post.tile([4, D], bf16, tag="vsM")
        nc.vector.tensor_scalar(
            tmpo[0:4, :], bv4, float(N_FULL), None, op0=mybir.AluOpType.mult
        )
        nc.vector.tensor_add(vsM, vs4, tmpo[0:4, :])
        # den lhsT tiles: ksM01/ksM23 [128, 4] = 4*ks placed block-diagonally
        ksM = post.tile([128, 2, 4], bf16, tag="ksM")
        ks4 = post.tile([128, 2, 1], f32, tag="ks4")
        nc.vector.tensor_scalar(
            ks4, ksc, 4.0, None, op0=mybir.AluOpType.mult
        )
        nc.vector.memset(ksM, 0.0)
        for j in range(2):
            for i in range(2):
                nc.vector.tensor_copy(
                    ksM[ds(i * 64, 64), j, 2 * j + i : 2 * j + i + 1],
                    ks4[ds(i * 64, 64), j, :],
                )
        # Ndiag [4,4] = 4N * I
        ndiag = post.tile([4, 4], bf16, tag="ndiag")
        nc.vector.tensor_scalar(
            ndiag, ident[0:4, 0:4], 4.0 * N_FULL, None, op0=mybir.AluOpType.mult
        )

        # ---- phase D: den -> rec -> bc -> scale -> num -> store -------
        for ci in range(NCH):
            w = chunk_width(ci)
            qc01 = qT_all[:, 0, ci, 0:w]
            qc23 = qT_all[:, 1, ci, 0:w]
            rT = rT_all[:, ci, 0:w]

            den_ps = ps_r2.tile([4, 512], f32, tag="r2", name=f"den_{ci}")
            nc.tensor.matmul(den_ps[:, 0:w], ksM[:, 0, :], qc01, start=True, stop=False)
            nc.tensor.matmul(den_ps[:, 0:w], ksM[:, 1, :], qc23, start=False, stop=False)
            nc.tensor.matmul(den_ps[:, 0:w], ndiag, rT, start=False, stop=True)
            rec = dpool.tile([4, 512], bf16, tag="rec", name=f"rec_{ci}")
            with nc.allow_low_precision(reason="bf16 1/den: 0.4% rel, budget 2e-2"):
                nc.vector.reciprocal(rec[:, 0:w], den_ps[:, 0:w])

            bc01_ps = ps_q01.tile([128, 512], f32, tag="qT01", name=f"bc01_{ci}")
            bc23_ps = ps_q23.tile([128, 512], f32, tag="qT23", name=f"bc23_{ci}")
            nc.tensor.matmul(bc01_ps[:, 0:w], sel01, rec[:, 0:w], start=True, stop=True)
            nc.tensor.matmul(bc23_ps[:, 0:w], sel23, rec[:, 0:w], start=True, stop=True)
            bc01 = dpool.tile([128, 512], bf16, tag="bc01", name=f"bc01s_{ci}")
            bc23 = dpool.tile([128, 512], bf16, tag="bc23", name=f"bc23s_{ci}")
            nc.scalar.copy(out=bc01[:, 0:w], in_=bc01_ps[:, 0:w])
            nc.scalar.copy(out=bc23[:, 0:w], in_=bc23_ps[:, 0:w])

            qs01 = dpool.tile([128, 512], bf16, tag="qs01", name=f"qs01_{ci}")
            qs23 = dpool.tile([128, 512], bf16, tag="qs23", name=f"qs23_{ci}")
            nc.vector.tensor_mul(qs01[:, 0:w], qc01, bc01[:, 0:w])
            nc.vector.tensor_mul(qs23[:, 0:w], qc23, bc23[:, 0:w])
            rs = dpool.tile([4, 512], bf16, tag="rs", name=f"rs_{ci}")
            nc.vector.tensor_mul(rs[:, 0:w], rT, rec[:, 0:w])

            o_ps = ps_kv.tile([64, 512], f32, tag="kv_ps", name=f"o_ps_{ci}")
            nc.tensor.matmul(o_ps[:, 0:w], lhs_bf[0], qs01[:, 0:w], start=True, stop=False)
            nc.tensor.matmul(o_ps[:, 0:w], lhs_bf[1], qs23[:, 0:w], start=False, stop=False)
            nc.tensor.matmul(o_ps[:, 0:w], vsM, rs[:, 0:w], start=False, stop=True)
            osb = opool.tile([64, 512], f32, tag="osb", name=f"osb_{ci}")
            nc.scalar.copy(out=osb[:, 0:w], in_=o_ps[:, 0:w])
            eng = nc.gpsimd if ci % 2 == 0 else nc.sync
            eng.dma_start(out=outT[:, ds(ci * 512, w)], in_=osb[:, 0:w])

    nc.finalize()
    return nc


def _get_nc():
    global _nc_cache
    if _nc_cache is None:
        _nc_cache = _build_nc()
    return _nc_cache


def kernel(x, Wq_w, Wq_b, Wk_w, Wk_b, Wv_w, Wv_b, n_nodes=None):
    import ml_dtypes
    from concourse.bass_utils import run_bass_kernel_spmd

    x = np.asarray(x, np.float32)
    xp = np.zeros((NCORES, NL, IN), np.float32)
    xp[:, :NLOC, :] = x.reshape(NCORES, NLOC, IN)

    wqT = np.ascontiguousarray(np.asarray(Wq_w, np.float32).T)
    wkT = np.ascontiguousarray(np.asarray(Wk_w, np.float32).T)
    wvT = np.ascontiguousarray(np.asarray(Wv_w, np.float32).T)
    bq = np.asarray(Wq_b, np.float32).reshape(1, HD)
    bk = np.asarray(Wk_b, np.float32).reshape(1, HD)
    bv = np.asarray(Wv_b, np.float32).reshape(1, HD)
    pm = np.ones((128, 1), np.float32)
    pm[PAD0:, 0] = 0.0
    import ml_dtypes as _md

    selc = np.zeros((8, 128), _md.bfloat16)
    selc[0, 0:64] = 1
    selc[1, 64:128] = 1
    selc[6, 0:64] = 1
    selc[7, 64:128] = 1

    nc = _get_nc()
    in_maps = []
    for c in range(NCORES):
        xTb = np.ascontiguousarray(xp[c].T).astype(ml_dtypes.bfloat16)
        in_maps.append(
            {
                "xT": xTb,
                "wqT": wqT,
                "wkT": wkT,
                "wvT": wvT,
                "bq": bq,
                "bk": bk,
                "bv": bv,
                "padmask": pm,
                "selc": selc,
            }
        )
    import os

    trace = bool(os.environ.get("KERNEL_TRACE"))
    stitch = bool(os.environ.get("KERNEL_TRACE_STITCH"))
    tcores = os.environ.get("KERNEL_TRACE_CORES", "0")
    trace_cores = [int(c) for c in tcores.split(",")] if trace else None
    res = run_bass_kernel_spmd(
        nc,
        in_maps,
        core_ids=list(range(NCORES)),
        trace=trace,
        trace_cores=trace_cores,
        stitch_traces=stitch,
    )
    global _last_result
    _last_result = res
    outs = [
        np.ascontiguousarray(res.results[c]["outT"].T[:NLOC, :])
        for c in range(NCORES)
    ]
    return np.concatenate(outs, axis=0).astype(np.float32)


if __name__ == "__main__":
    rng = np.random.default_rng(0)
    s = 1.0 / np.sqrt(IN)
    inputs = {
        "x": rng.standard_normal((N_FULL, IN)).astype(np.float32),
        "Wq_w": rng.uniform(-s, s, (HD, IN)).astype(np.float32),
        "Wq_b": rng.uniform(-s, s, HD).astype(np.float32),
        "Wk_w": rng.uniform(-s, s, (HD, IN)).astype(np.float32),
        "Wk_b": rng.uniform(-s, s, HD).astype(np.float32),
        "Wv_w": rng.uniform(-s, s, (HD, IN)).astype(np.float32),
        "Wv_b": rng.uniform(-s, s, HD).astype(np.float32),
        "n_nodes": np.array([N_FULL], np.int32),
    }
    o = kernel(**inputs)
    print(o.shape, o.dtype, np.abs(o).max())


# revision 22
# speedup vs baseline: 1.6527x; 1.6527x over previous
# DIFFormerConv (linear attention) Trainium2 kernel — 8-core data-parallel.
#
# Math (per head h, D=64, H=4, N nodes):
#   q = x@Wq^T + bq ; k = x@Wk^T + bk ; v = x@Wv^T + bv
#   qn = q/|q|, kn = k/|k|  (row-wise per head)
#   kvs = sum_l kn_l (x) v_l   [H,D,D];  vs = sum_l v_l;  ks = sum_l kn_l
#   out = mean_h (qn@kvs + vs) / (qn.ks + N)
#
# Key structure (stage 2 — transpose-free):
#   * q is computed TRANSPOSED directly on the PE: qT = Wq @ x^T with
#     lhsT = wqT column blocks and rhs = the (already transposed) xT
#     input. Two heads stack per PSUM tile ([128, nodes] = h0 rows 0:64,
#     h1 rows 64:128). This removes every DMA-transpose the previous
#     design needed, at identical PE stream cost.
#   * r = |q| per head rides as a separate [4, nodes] row tile, computed
#     by PE block-ones matmuls over q^2 (partition-dim reduction).
#   * q-normalization cancels in the ratio:
#       (qn@kvs+vs)/(qn.ks+N) == (q@kvs + r*vs)/(q.ks + r*N)
#   * per-node denominators den = q.(4ks) + r.4N come from 3 small PE
#     matmuls in transposed space; 1/den is broadcast back to 128
#     partitions with a selector matmul, and q is scaled by it with
#     bf16 DVE muls. The final num matmul contracts head PAIRS at once
#     (lhsT = stacked kvs), plus one [4,w] matmul for the r*vs term.
#   * v-bias deferred: kvs += ks (x) bv, vs += N*bv (post-AllReduce).
#   * the AllReduce payload is packed to [33,512] f32 (68KB).
#   * output is written transposed ([64, NL]) and flipped on the host.

import numpy as np

N_FULL = 100000
IN = 256
D = 64
H = 4
HD = 256
NCORES = 8
NLOC = N_FULL // NCORES          # 12500
NT = 98                          # l-tiles of 128 (padded)
NL = NT * 128                    # 12544
NG = NT // 2                     # 49 groups of 256 nodes
PAD0 = NLOC - (NT - 1) * 128     # used rows in last tile = 84
NCH = 25                         # chunks of 512 nodes (last = 256)

_nc_cache = None
_last_result = None


def _build_nc():
    from contextlib import ExitStack

    import concourse.bass as bass
    import concourse.mybir as mybir
    import concourse.tile as tile
    from concourse import bacc
    from concourse.bass import ds
    from concourse.masks import make_identity
    from bass_rust import add_dep_helper

    f32 = mybir.dt.float32
    bf16 = mybir.dt.bfloat16

    nc = bacc.Bacc()

    xT = nc.dram_tensor("xT", [IN, NL], bf16, kind="ExternalInput")
    wqT = nc.dram_tensor("wqT", [IN, HD], f32, kind="ExternalInput")
    wkT = nc.dram_tensor("wkT", [IN, HD], f32, kind="ExternalInput")
    wvT = nc.dram_tensor("wvT", [IN, HD], f32, kind="ExternalInput")
    bq = nc.dram_tensor("bq", [1, HD], f32, kind="ExternalInput")
    bk = nc.dram_tensor("bk", [1, HD], f32, kind="ExternalInput")
    bv = nc.dram_tensor("bv", [1, HD], f32, kind="ExternalInput")
    padmask = nc.dram_tensor("padmask", [128, 1], f32, kind="ExternalInput")
    outT = nc.dram_tensor("outT", [D, NL], f32, kind="ExternalOutput")

    with tile.TileContext(nc) as tc, ExitStack() as ctx:
        consts = ctx.enter_context(tc.tile_pool(name="consts", bufs=1))
        xtpool = ctx.enter_context(tc.tile_pool(name="xtpool", bufs=3))
        knvpool = ctx.enter_context(tc.tile_pool(name="knvpool", bufs=3))
        spool = ctx.enter_context(tc.tile_pool(name="spool", bufs=3))
        qapool = ctx.enter_context(tc.tile_pool(name="qapool", bufs=1))
        post = ctx.enter_context(tc.tile_pool(name="post", bufs=1))
        dpool = ctx.enter_context(tc.tile_pool(name="dpool", bufs=3))
        opool = ctx.enter_context(tc.tile_pool(name="opool", bufs=3))

        ps_kv = ctx.enter_context(tc.tile_pool(name="ps_kv", bufs=1, space="PSUM"))
        ps_q01 = ctx.enter_context(tc.tile_pool(name="ps_q01", bufs=1, space="PSUM"))
        ps_q23 = ctx.enter_context(tc.tile_pool(name="ps_q23", bufs=1, space="PSUM"))
        ps_acc = ctx.enter_context(tc.tile_pool(name="ps_acc", bufs=1, space="PSUM"))
        dram = ctx.enter_context(tc.tile_pool(name="dram", bufs=1, space="DRAM"))

        # ---- constants -------------------------------------------------
        # wq kept separate (used as [128,128] head-pair column blocks);
        # wk|wv merged so one N=512 matmul produces k and v together.
        wq_sb = consts.tile([128, 2, HD], bf16, tag="wq_sb")
        nc.gpsimd.dma_start(
            out=wq_sb, in_=wqT[:, :].rearrange("(cb p) f -> p cb f", p=128)
        )
        wkv_sb = consts.tile([128, 2, 512], bf16, tag="wkv_sb")
        nc.gpsimd.dma_start(
            out=wkv_sb[:, :, 0:HD],
            in_=wkT[:, :].rearrange("(cb p) f -> p cb f", p=128),
        )
        nc.gpsimd.dma_start(
            out=wkv_sb[:, :, ds(HD, HD)],
            in_=wvT[:, :].rearrange("(cb p) f -> p cb f", p=128),
        )

        bq_sb = consts.tile([1, HD], bf16, tag="bq")
        nc.gpsimd.dma_start(out=bq_sb, in_=bq[:, :])
        # k bias padded with zeros on the v half (v-bias deferred)
        bkv_sb = consts.tile([1, 512], bf16, tag="bkv")
        nc.vector.memset(bkv_sb, 0.0)
        nc.gpsimd.dma_start(out=bkv_sb[:, 0:HD], in_=bk[:, :])

        # bv as [4, 64] rows (per head) and stacked pair columns
        bv4 = consts.tile([4, D], f32, tag="bv4")
        nc.gpsimd.dma_start(out=bv4, in_=bv[:, :])
        bv01_bc = consts.tile([128, D], f32, tag="bv01_bc")
        bv23_bc = consts.tile([128, D], f32, tag="bv23_bc")
        for j, tgt in ((0, bv01_bc), (2, bv23_bc)):
            for i in range(2):
                h = j + i
                src = bv[:, ds(h * 64, 64)]
                nc.gpsimd.dma_start(
                    out=tgt[ds(i * 64, 64), :],
                    in_=bass.AP(
                        tensor=src.tensor, offset=src.offset,
                        ap=[[0, 64]] + src.ap[1:],
                    ),
                )
        padmask_sb = consts.tile([128, 1], f32, tag="padmask_sb")
        nc.sync.dma_start(out=padmask_sb, in_=padmask[:, :])

        ones_row = consts.tile([1, 512], bf16, tag="ones_row")
        nc.vector.memset(ones_row, 1.0)
        ones_col = consts.tile([128, 1], bf16, tag="ones_col")
        nc.vector.memset(ones_col, 1.0)
        ident = consts.tile([64, 64], f32, tag="ident")
        make_identity(nc, ident)

        # block-ones selectors for partition-dim head reductions
        # boA: cols 0,1 = ones on partitions 0:64 / 64:128 (for sq01)
        # boB: cols 2,3 = ones on partitions 0:64 / 64:128 (for sq23)
        boA = consts.tile([128, 4], bf16, tag="boA")
        boB = consts.tile([128, 4], bf16, tag="boB")
        nc.vector.memset(boA, 0.0)
        nc.vector.memset(boB, 0.0)
        nc.vector.memset(boA[0:64, 0:1], 1.0)
        nc.vector.memset(boA[64:128, 1:2], 1.0)
        nc.vector.memset(boB[0:64, 2:3], 1.0)
        nc.vector.memset(boB[64:128, 3:4], 1.0)
        # selectors broadcasting rec rows back to head-pair partitions
        # (host-provided: DVE memset can't target partition bases 1..3)
        selc = nc.dram_tensor("selc", [8, 128], bf16, kind="ExternalInput")
        sel01 = consts.tile([4, 128], bf16, tag="sel01")
        sel23 = consts.tile([4, 128], bf16, tag="sel23")
        nc.gpsimd.dma_start(out=sel01, in_=selc[0:4, :])
        nc.gpsimd.dma_start(out=sel23, in_=selc[4:8, :])

        # persistent transposed-q store + per-head norms
        qT_all = qapool.tile([128, 2, NCH, 512], bf16, tag="qT_all")
        rT_all = qapool.tile([4, NCH, 512], bf16, tag="rT_all")

        # kvs accumulators: kn01^T @ [v | 1] and kn23^T @ [v | 1]
        kv01_acc = ps_acc.tile([128, 257], f32, tag="kv01_acc")
        kv23_acc = ps_acc.tile([128, 257], f32, tag="kv23_acc")
        # one shared bank: rows 0:4 = per-chunk r2 (and phase-D den),
        # row 64 = persistent vs accumulator
        r2vs = ps_acc.tile([65, 512], f32, tag="r2vs")
        vs_acc = r2vs[64:65, 0:257]

        def chunk_width(ci):
            return 512 if ci < NCH - 1 else 256

        # ---- main loop: chunks of 512 nodes (2 groups of 256) ---------
        for ci in range(NCH):
            w = chunk_width(ci)
            ngr = w // 256
            xt_g = xtpool.tile([128, 2, 512], bf16, tag="xt_g", name=f"xt_{ci}")
            nc.sync.dma_start(
                out=xt_g[:, :, 0:w],
                in_=xT[:, ds(ci * 512, w)].rearrange("(cb c) l -> c cb l", c=128),
            )

            # --- qT: two head-pair stacks, bias via ones_row ---
            qT01_ps = ps_q01.tile([128, 512], f32, tag="qT01", name=f"qT01_{ci}")
            qT23_ps = ps_q23.tile([128, 512], f32, tag="qT23", name=f"qT23_{ci}")
            for pair, q_ps in ((0, qT01_ps), (1, qT23_ps)):
                for cb in range(2):
                    nc.tensor.matmul(
                        q_ps[:, 0:w],
                        wq_sb[:, cb, ds(pair * 128, 128)],
                        xt_g[:, cb, 0:w],
                        start=(cb == 0),
                        stop=False,
                    )
                nc.tensor.matmul(
                    q_ps[:, 0:w],
                    bq_sb[:, ds(pair * 128, 128)],
                    ones_row[:, 0:w],
                    start=False,
                    stop=True,
                )
            qc01 = qT_all[:, 0, ci, :]
            qc23 = qT_all[:, 1, ci, :]
            nc.scalar.copy(out=qc01[:, 0:w], in_=qT01_ps[:, 0:w])
            nc.vector.tensor_copy(qc23[:, 0:w], qT23_ps[:, 0:w])

            # --- r2 = per-head column sums of q^2 (PE block-ones) ---
            # sq on gpsimd: off the critical chain (only consumed post-AR)
            sq01 = spool.tile([128, 512], bf16, tag="sq01", name=f"sq01_{ci}")
            sq23 = spool.tile([128, 512], bf16, tag="sq23", name=f"sq23_{ci}")
            nc.gpsimd.tensor_mul(sq01[:, 0:w], qc01[:, 0:w], qc01[:, 0:w])
            nc.gpsimd.tensor_mul(sq23[:, 0:w], qc23[:, 0:w], qc23[:, 0:w])
            nc.tensor.matmul(
                r2vs[0:4, 0:w], boA, sq01[:, 0:w], start=True, stop=False
            )
            nc.tensor.matmul(
                r2vs[0:4, 0:w], boB, sq23[:, 0:w], start=False, stop=True
            )
            nc.scalar.sqrt(rT_all[:, ci, 0:w], r2vs[0:4, 0:w])

            # --- k,v (normal layout) + kn + kvs, per 256-node group ---
            for gi in range(ngr):
                g = 2 * ci + gi
                kv_ps = ps_kv.tile(
                    [128, 2, 512], f32, tag="kv_ps", name=f"kv_ps_{g}"
                )
                for t in range(2):
                    lhs0 = xt_g[:, 0, ds(gi * 256 + t * 128, 128)]
                    lhs1 = xt_g[:, 1, ds(gi * 256 + t * 128, 128)]
                    nc.tensor.matmul(
                        kv_ps[:, t, :], lhs0, wkv_sb[:, 0, :], start=True, stop=False
                    )
                    nc.tensor.matmul(
                        kv_ps[:, t, :], lhs1, wkv_sb[:, 1, :], start=False, stop=False
                    )
                    nc.tensor.matmul(
                        kv_ps[:, t, :], ones_row[:, 0:128], bkv_sb,
                        start=False, stop=True,
                    )

                # one evacuation for k AND v; col 512 = ones (ks column)
                kvbf = spool.tile([128, 2, 513], bf16, tag="kvbf", name=f"kvbf_{g}")
                nc.vector.tensor_copy(kvbf[:, :, 0:512], kv_ps)
                nc.vector.memset(kvbf[:, :, 512:513], 1.0)
                kbf4 = kvbf[:, :, 0:HD].rearrange("p t (h d) -> p t h d", h=H)
                sqk = spool.tile([128, 2, H, 64], bf16, tag="sqk", name=f"sqk_{g}")
                nc.vector.tensor_mul(sqk, kbf4, kbf4)
                rk2 = spool.tile([128, 2, H, 1], f32, tag="rk2", name=f"rk2_{g}")
                nc.vector.tensor_reduce(
                    rk2, sqk, axis=mybir.AxisListType.X, op=mybir.AluOpType.add
                )
                rk = spool.tile([128, 2, H, 1], f32, tag="rk", name=f"rk_{g}")
                nc.scalar.sqrt(rk, rk2)
                wk_s = spool.tile([128, 2, H, 1], f32, tag="wk_s", name=f"wk_s_{g}")
                nc.vector.reciprocal(wk_s, rk)

                knv = knvpool.tile([128, 2, HD], bf16, tag="knv", name=f"knv_{g}")
                knv4 = knv.rearrange("p t (h d) -> p t h d", h=H)
                nc.vector.tensor_mul(
                    knv4, kbf4, wk_s[:, :, :, :].to_broadcast([128, 2, H, 64])
                )
                if g == NG - 1:
                    nc.vector.tensor_scalar_mul(
                        knv[:, 1, :], knv[:, 1, :], padmask_sb
                    )

                first = g == 0
                last = g == NG - 1
                for t in range(2):
                    st = first and t == 0
                    sp = last and t == 1
                    nc.tensor.matmul(
                        kv01_acc, knv[:, t, 0:128], kvbf[:, t, ds(HD, 257)],
                        start=st, stop=sp,
                    )
                    nc.tensor.matmul(
                        kv23_acc, knv[:, t, ds(128, 128)], kvbf[:, t, ds(HD, 257)],
                        start=st, stop=sp,
                    )
                    nc.tensor.matmul(
                        vs_acc, ones_col, kvbf[:, t, ds(HD, 257)],
                        start=st, stop=sp,
                    )

        # ---- pack + AllReduce ([33,512] f32 = 68KB) -------------------
        # rows 0:32 = kvs blocks (as [64,256]: col-block h = kvs_h),
        # row 32 = [ks 256 | vs 256]
        kv01_sb = post.tile([128, 257], f32, tag="kv01_sb")
        kv23_sb = post.tile([128, 257], f32, tag="kv23_sb")
        vs_sb = post.tile([1, 257], f32, tag="vs_sb")
        nc.vector.tensor_copy(kv01_sb, kv01_acc)
        nc.vector.tensor_copy(kv23_sb, kv23_acc)
        nc.vector.tensor_copy(vs_sb, vs_acc)

        ar_in = dram.tile([33, 512], f32, tag="ar_in")
        ar_out = dram.tile([33, 512], f32, tag="ar_out")
        arv = ar_in[0:32, :].rearrange("p (a c) -> (p a) c", a=2)  # [64, 256]
        blocks = [
            (kv01_sb, 0, 0), (kv01_sb, 64, 64),
            (kv23_sb, 0, 128), (kv23_sb, 64, 192),
        ]
        ar_dmas = []
        for h, (src, r0, c0) in enumerate(blocks):
            ar_dmas.append(
                nc.sync.dma_start(
                    out=arv[:, ds(h * 64, 64)], in_=src[ds(r0, 64), ds(c0, 64)]
                )
            )
            ar_dmas.append(
                nc.scalar.dma_start(
                    out=ar_in[32:33, ds(h * 64, 64)], in_=src[ds(r0, 64), 256:257]
                )
            )
        ar_dmas.append(
            nc.sync.dma_start(out=ar_in[32:33, ds(HD, HD)], in_=vs_sb[:, 0:HD])
        )
        pnop = nc.gpsimd.nop()
        for dd in ar_dmas:
            add_dep_helper(pnop.ins, dd.ins, sync=True, reason="absorb ar_in dma")
        nc.gpsimd.collective_compute(
            "AllReduce",
            mybir.AluOpType.add,
            ins=[ar_in[:, :].opt()],
            outs=[ar_out[:, :].opt()],
            replica_groups=[list(range(NCORES))],
        )

        # ---- post-reduce fixups ---------------------------------------
        # sums row
        sumr = post.tile([1, 512], f32, tag="sumr")
        nc.sync.dma_start(out=sumr, in_=ar_out[32:33, :])
        arov = ar_out[0:32, :].rearrange("p (a c) -> (p a) c", a=2)  # [64, 256]

        # stacked kvs lhsT tiles [128, 64]: rows 0:64 = kvs_h(even),
        # 64:128 = kvs_h(odd); then += ks (x) bv and cast to bf16
        kvstk = []
        for j in range(2):  # pair 01, 23
            stk = post.tile([128, D], f32, tag=f"kvstk{j}")
            nc.sync.dma_start(out=stk[0:64, :], in_=arov[:, ds((2 * j) * 64, 64)])
            nc.sync.dma_start(
                out=stk[64:128, :], in_=arov[:, ds((2 * j + 1) * 64, 64)]
            )
            kvstk.append(stk)
        # ks columns stacked to match: [128,1] per pair
        ksc = post.tile([128, 2, 1], f32, tag="ksc")
        for j in range(2):
            for i in range(2):
                h = 2 * j + i
                nc.sync.dma_start(
                    out=ksc[ds(i * 64, 64), j, :],
                    in_=ar_out[32:33, ds(h * 64, 64)],
                )
        tmpo = post.tile([128, D], f32, tag="tmpo")
        lhs_bf = []
        for j, bvbc in ((0, bv01_bc), (1, bv23_bc)):
            nc.vector.tensor_scalar_mul(tmpo, bvbc, ksc[:, j, :])
            lb = post.tile([128, D], bf16, tag=f"lhsbf{j}")
            nc.vector.tensor_add(lb, kvstk[j], tmpo)
            lhs_bf.append(lb)
        # vs rows [4, 64] + N*bv, bf16
        vs4 = post.tile([4, D], f32, tag="vs4")
        nc.sync.dma_start(out=vs4, in_=ar_out[32:33, ds(HD, HD)])
        vsM = post.tile([4, D], bf16, tag="vsM")
        nc.vector.tensor_scalar(
            tmpo[0:4, :], bv4, float(N_FULL), None, op0=mybir.AluOpType.mult
        )
        nc.vector.tensor_add(vsM, vs4, tmpo[0:4, :])
        # den lhsT tiles: ksM01/ksM23 [128, 4] = 4*ks placed block-diagonally
        ksM = post.tile([128, 2, 4], bf16, tag="ksM")
        ks4 = post.tile([128, 2, 1], f32, tag="ks4")
        nc.vector.tensor_scalar(
            ks4, ksc, 4.0, None, op0=mybir.AluOpType.mult
        )
        nc.vector.memset(ksM, 0.0)
        for j in range(2):
            for i in range(2):
                nc.vector.tensor_copy(
                    ksM[ds(i * 64, 64), j, 2 * j + i : 2 * j + i + 1],
                    ks4[ds(i * 64, 64), j, :],
                )
        # Ndiag [4,4] = 4N * I
        ndiag = post.tile([4, 4], bf16, tag="ndiag")
        nc.vector.tensor_scalar(
            ndiag, ident[0:4, 0:4], 4.0 * N_FULL, None, op0=mybir.AluOpType.mult
        )

        # ---- phase D: den -> rec -> bc -> scale -> num -> store -------
        for ci in range(NCH):
            w = chunk_width(ci)
            qc01 = qT_all[:, 0, ci, 0:w]
            qc23 = qT_all[:, 1, ci, 0:w]
            rT = rT_all[:, ci, 0:w]

            den_ps = r2vs[0:4, :]
            nc.tensor.matmul(den_ps[:, 0:w], ksM[:, 0, :], qc01, start=True, stop=False)
            nc.tensor.matmul(den_ps[:, 0:w], ksM[:, 1, :], qc23, start=False, stop=False)
            nc.tensor.matmul(den_ps[:, 0:w], ndiag, rT, start=False, stop=True)
            rec = dpool.tile([4, 512], bf16, tag="rec", name=f"rec_{ci}")
            with nc.allow_low_precision(reason="bf16 1/den: 0.4% rel, budget 2e-2"):
                nc.vector.reciprocal(rec[:, 0:w], den_ps[:, 0:w])

            bc01_ps = ps_q01.tile([128, 512], f32, tag="qT01", name=f"bc01_{ci}")
            bc23_ps = ps_q23.tile([128, 512], f32, tag="qT23", name=f"bc23_{ci}")
            nc.tensor.matmul(bc01_ps[:, 0:w], sel01, rec[:, 0:w], start=True, stop=True)
            nc.tensor.matmul(bc23_ps[:, 0:w], sel23, rec[:, 0:w], start=True, stop=True)
            bc01 = dpool.tile([128, 512], bf16, tag="bc01", name=f"bc01s_{ci}")
            bc23 = dpool.tile([128, 512], bf16, tag="bc23", name=f"bc23s_{ci}")
            nc.scalar.copy(out=bc01[:, 0:w], in_=bc01_ps[:, 0:w])
            nc.scalar.copy(out=bc23[:, 0:w], in_=bc23_ps[:, 0:w])

            qs01 = dpool.tile([128, 512], bf16, tag="qs01", name=f"qs01_{ci}")
            qs23 = dpool.tile([128, 512], bf16, tag="qs23", name=f"qs23_{ci}")
            nc.vector.tensor_mul(qs01[:, 0:w], qc01, bc01[:, 0:w])
            nc.vector.tensor_mul(qs23[:, 0:w], qc23, bc23[:, 0:w])
            rs = dpool.tile([4, 512], bf16, tag="rs", name=f"rs_{ci}")
            nc.vector.tensor_mul(rs[:, 0:w], rT, rec[:, 0:w])

            o_ps = ps_kv.tile([64, 512], f32, tag="kv_ps", name=f"o_ps_{ci}")
            nc.tensor.matmul(o_ps[:, 0:w], lhs_bf[0], qs01[:, 0:w], start=True, stop=False)
            nc.tensor.matmul(o_ps[:, 0:w], lhs_bf[1], qs23[:, 0:w], start=False, stop=False)
            nc.tensor.matmul(o_ps[:, 0:w], vsM, rs[:, 0:w], start=False, stop=True)
            osb = opool.tile([64, 512], f32, tag="osb", name=f"osb_{ci}")
            nc.scalar.copy(out=osb[:, 0:w], in_=o_ps[:, 0:w])
            eng = nc.gpsimd if ci % 2 == 0 else nc.sync
            eng.dma_start(out=outT[:, ds(ci * 512, w)], in_=osb[:, 0:w])

    nc.finalize()
    return nc


def _get_nc():
    global _nc_cache
    if _nc_cache is None:
        _nc_cache = _build_nc()
    return _nc_cache


def kernel(x, Wq_w, Wq_b, Wk_w, Wk_b, Wv_w, Wv_b, n_nodes=None):
    import ml_dtypes
    from concourse.bass_utils import run_bass_kernel_spmd

    x = np.asarray(x, np.float32)
    xp = np.zeros((NCORES, NL, IN), np.float32)
    xp[:, :NLOC, :] = x.reshape(NCORES, NLOC, IN)

    wqT = np.ascontiguousarray(np.asarray(Wq_w, np.float32).T)
    wkT = np.ascontiguousarray(np.asarray(Wk_w, np.float32).T)
    wvT = np.ascontiguousarray(np.asarray(Wv_w, np.float32).T)
    bq = np.asarray(Wq_b, np.float32).reshape(1, HD)
    bk = np.asarray(Wk_b, np.float32).reshape(1, HD)
    bv = np.asarray(Wv_b, np.float32).reshape(1, HD)
    pm = np.ones((128, 1), np.float32)
    pm[PAD0:, 0] = 0.0
    import ml_dtypes as _md

    selc = np.zeros((8, 128), _md.bfloat16)
    selc[0, 0:64] = 1
    selc[1, 64:128] = 1
    selc[6, 0:64] = 1
    selc[7, 64:128] = 1

    nc = _get_nc()
    in_maps = []
    for c in range(NCORES):
        xTb = np.ascontiguousarray(xp[c].T).astype(ml_dtypes.bfloat16)
        in_maps.append(
            {
                "xT": xTb,
                "wqT": wqT,
                "wkT": wkT,
                "wvT": wvT,
                "bq": bq,
                "bk": bk,
                "bv": bv,
                "padmask": pm,
                "selc": selc,
            }
        )
    import os

    trace = bool(os.environ.get("KERNEL_TRACE"))
    stitch = bool(os.environ.get("KERNEL_TRACE_STITCH"))
    tcores = os.environ.get("KERNEL_TRACE_CORES", "0")
    trace_cores = [int(c) for c in tcores.split(",")] if trace else None
    res = run_bass_kernel_spmd(
        nc,
        in_maps,
        core_ids=list(range(NCORES)),
        trace=trace,
        trace_cores=trace_cores,
        stitch_traces=stitch,
    )
    global _last_result
    _last_result = res
    outs = [
        np.ascontiguousarray(res.results[c]["outT"].T[:NLOC, :])
        for c in range(NCORES)
    ]
    return np.concatenate(outs, axis=0).astype(np.float32)


if __name__ == "__main__":
    rng = np.random.default_rng(0)
    s = 1.0 / np.sqrt(IN)
    inputs = {
        "x": rng.standard_normal((N_FULL, IN)).astype(np.float32),
        "Wq_w": rng.uniform(-s, s, (HD, IN)).astype(np.float32),
        "Wq_b": rng.uniform(-s, s, HD).astype(np.float32),
        "Wk_w": rng.uniform(-s, s, (HD, IN)).astype(np.float32),
        "Wk_b": rng.uniform(-s, s, HD).astype(np.float32),
        "Wv_w": rng.uniform(-s, s, (HD, IN)).astype(np.float32),
        "Wv_b": rng.uniform(-s, s, HD).astype(np.float32),
        "n_nodes": np.array([N_FULL], np.int32),
    }
    o = kernel(**inputs)
    print(o.shape, o.dtype, np.abs(o).max())


# revision 25
# speedup vs baseline: 2.0096x; 1.2159x over previous
# DIFFormerConv (linear attention) Trainium2 kernel — 8-core data-parallel.
#
# Math (per head h, D=64, H=4, N nodes):
#   q = x@Wq^T + bq ; k = x@Wk^T + bk ; v = x@Wv^T + bv
#   qn = q/|q|, kn = k/|k|  (row-wise per head)
#   kvs = sum_l kn_l (x) v_l   [H,D,D];  vs = sum_l v_l;  ks = sum_l kn_l
#   out = mean_h (qn@kvs + vs) / (qn.ks + N)
#
# Key structure (stage 2 — transpose-free):
#   * q is computed TRANSPOSED directly on the PE: qT = Wq @ x^T with
#     lhsT = wqT column blocks and rhs = the (already transposed) xT
#     input. Two heads stack per PSUM tile ([128, nodes] = h0 rows 0:64,
#     h1 rows 64:128). This removes every DMA-transpose the previous
#     design needed, at identical PE stream cost.
#   * r = |q| per head rides as a separate [4, nodes] row tile, computed
#     by PE block-ones matmuls over q^2 (partition-dim reduction).
#   * q-normalization cancels in the ratio:
#       (qn@kvs+vs)/(qn.ks+N) == (q@kvs + r*vs)/(q.ks + r*N)
#   * per-node denominators den = q.(4ks) + r.4N come from 3 small PE
#     matmuls in transposed space; 1/den is broadcast back to 128
#     partitions with a selector matmul, and q is scaled by it with
#     bf16 DVE muls. The final num matmul contracts head PAIRS at once
#     (lhsT = stacked kvs), plus one [4,w] matmul for the r*vs term.
#   * v-bias deferred: kvs += ks (x) bv, vs += N*bv (post-AllReduce).
#   * the AllReduce payload is packed to [33,512] f32 (68KB).
#   * output is written transposed ([64, NL]) and flipped on the host.

import numpy as np

N_FULL = 100000
IN = 256
D = 64
H = 4
HD = 256
NCORES = 8
NLOC = N_FULL // NCORES          # 12500
NT = 98                          # l-tiles of 128 (padded)
NL = NT * 128                    # 12544
NG = NT // 2                     # 49 groups of 256 nodes
PAD0 = NLOC - (NT - 1) * 128     # used rows in last tile = 84
NCH = 25                         # chunks of 512 nodes (last = 256)

_nc_cache = None
_last_result = None


def _build_nc():
    from contextlib import ExitStack

    import concourse.bass as bass
    import concourse.mybir as mybir
    import concourse.tile as tile
    from concourse import bacc
    from concourse.bass import ds
    from concourse.masks import make_identity
    from bass_rust import add_dep_helper

    f32 = mybir.dt.float32
    bf16 = mybir.dt.bfloat16

    nc = bacc.Bacc()

    xT = nc.dram_tensor("xT", [IN, NL], bf16, kind="ExternalInput")
    wqT = nc.dram_tensor("wqT", [IN, HD], f32, kind="ExternalInput")
    wkT = nc.dram_tensor("wkT", [IN, HD], f32, kind="ExternalInput")
    wvT = nc.dram_tensor("wvT", [IN, HD], f32, kind="ExternalInput")
    bq = nc.dram_tensor("bq", [1, HD], f32, kind="ExternalInput")
    bk = nc.dram_tensor("bk", [1, HD], f32, kind="ExternalInput")
    bv = nc.dram_tensor("bv", [1, HD], f32, kind="ExternalInput")
    padmask = nc.dram_tensor("padmask", [128, 1], f32, kind="ExternalInput")
    outT = nc.dram_tensor("outT", [D, NL], f32, kind="ExternalOutput")

    with tile.TileContext(nc) as tc, ExitStack() as ctx:
        consts = ctx.enter_context(tc.tile_pool(name="consts", bufs=1))
        xtpool = ctx.enter_context(tc.tile_pool(name="xtpool", bufs=3))
        knvpool = ctx.enter_context(tc.tile_pool(name="knvpool", bufs=3))
        spool = ctx.enter_context(tc.tile_pool(name="spool", bufs=3))
        qapool = ctx.enter_context(tc.tile_pool(name="qapool", bufs=1))
        post = ctx.enter_context(tc.tile_pool(name="post", bufs=1))
        dpool = ctx.enter_context(tc.tile_pool(name="dpool", bufs=3))
        opool = ctx.enter_context(tc.tile_pool(name="opool", bufs=3))

        ps_kv = ctx.enter_context(tc.tile_pool(name="ps_kv", bufs=1, space="PSUM"))
        ps_q01 = ctx.enter_context(tc.tile_pool(name="ps_q01", bufs=1, space="PSUM"))
        ps_q23 = ctx.enter_context(tc.tile_pool(name="ps_q23", bufs=1, space="PSUM"))
        ps_acc = ctx.enter_context(tc.tile_pool(name="ps_acc", bufs=1, space="PSUM"))
        dram = ctx.enter_context(tc.tile_pool(name="dram", bufs=1, space="DRAM"))

        # ---- constants -------------------------------------------------
        # wq kept separate (used as [128,128] head-pair column blocks);
        # wk|wv merged so one N=512 matmul produces k and v together.
        wq_sb = consts.tile([128, 2, HD], bf16, tag="wq_sb")
        nc.gpsimd.dma_start(
            out=wq_sb, in_=wqT[:, :].rearrange("(cb p) f -> p cb f", p=128)
        )
        wkv_sb = consts.tile([128, 2, 512], bf16, tag="wkv_sb")
        nc.gpsimd.dma_start(
            out=wkv_sb[:, :, 0:HD],
            in_=wkT[:, :].rearrange("(cb p) f -> p cb f", p=128),
        )
        nc.gpsimd.dma_start(
            out=wkv_sb[:, :, ds(HD, HD)],
            in_=wvT[:, :].rearrange("(cb p) f -> p cb f", p=128),
        )

        bq_sb = consts.tile([1, HD], bf16, tag="bq")
        nc.gpsimd.dma_start(out=bq_sb, in_=bq[:, :])
        # k bias padded with zeros on the v half (v-bias deferred)
        bkv_sb = consts.tile([1, 512], bf16, tag="bkv")
        nc.vector.memset(bkv_sb, 0.0)
        nc.gpsimd.dma_start(out=bkv_sb[:, 0:HD], in_=bk[:, :])

        # bv as [4, 64] rows (per head) and stacked pair columns
        bv4 = consts.tile([4, D], f32, tag="bv4")
        nc.gpsimd.dma_start(out=bv4, in_=bv[:, :])
        bv01_bc = consts.tile([128, D], f32, tag="bv01_bc")
        bv23_bc = consts.tile([128, D], f32, tag="bv23_bc")
        for j, tgt in ((0, bv01_bc), (2, bv23_bc)):
            for i in range(2):
                h = j + i
                src = bv[:, ds(h * 64, 64)]
                nc.gpsimd.dma_start(
                    out=tgt[ds(i * 64, 64), :],
                    in_=bass.AP(
                        tensor=src.tensor, offset=src.offset,
                        ap=[[0, 64]] + src.ap[1:],
                    ),
                )
        padmask_sb = consts.tile([128, 1], f32, tag="padmask_sb")
        nc.sync.dma_start(out=padmask_sb, in_=padmask[:, :])

        ones_row = consts.tile([1, 512], bf16, tag="ones_row")
        nc.vector.memset(ones_row, 1.0)
        ones_col = consts.tile([128, 1], bf16, tag="ones_col")
        nc.vector.memset(ones_col, 1.0)
        ident = consts.tile([64, 64], f32, tag="ident")
        make_identity(nc, ident)

        # block-ones selectors for partition-dim head reductions
        # boA: cols 0,1 = ones on partitions 0:64 / 64:128 (for sq01)
        # boB: cols 2,3 = ones on partitions 0:64 / 64:128 (for sq23)
        boA = consts.tile([128, 4], bf16, tag="boA")
        boB = consts.tile([128, 4], bf16, tag="boB")
        nc.vector.memset(boA, 0.0)
        nc.vector.memset(boB, 0.0)
        nc.vector.memset(boA[0:64, 0:1], 1.0)
        nc.vector.memset(boA[64:128, 1:2], 1.0)
        nc.vector.memset(boB[0:64, 2:3], 1.0)
        nc.vector.memset(boB[64:128, 3:4], 1.0)
        # selectors broadcasting rec rows back to head-pair partitions
        # (host-provided: DVE memset can't target partition bases 1..3)
        selc = nc.dram_tensor("selc", [8, 128], bf16, kind="ExternalInput")
        sel01 = consts.tile([4, 128], bf16, tag="sel01")
        sel23 = consts.tile([4, 128], bf16, tag="sel23")
        nc.gpsimd.dma_start(out=sel01, in_=selc[0:4, :])
        nc.gpsimd.dma_start(out=sel23, in_=selc[4:8, :])

        # persistent transposed-q store + per-head norms
        qT_all = qapool.tile([128, 2, NCH, 512], bf16, tag="qT_all")
        rT_all = qapool.tile([4, NCH, 512], bf16, tag="rT_all")

        # kvs accumulators: kn01^T @ [v | 1] and kn23^T @ [v | 1]
        kv01_acc = ps_acc.tile([128, 257], f32, tag="kv01_acc")
        kv23_acc = ps_acc.tile([128, 257], f32, tag="kv23_acc")
        # one shared bank: rows 0:4 = per-chunk r2 (and phase-D den),
        # row 64 = persistent vs accumulator
        r2vs = ps_acc.tile([65, 512], f32, tag="r2vs")
        vs_acc = r2vs[64:65, 0:257]

        def chunk_width(ci):
            return 512 if ci < NCH - 1 else 256

        # ---- main loop: chunks of 512 nodes (2 groups of 256) ---------
        for ci in range(NCH):
            w = chunk_width(ci)
            ngr = w // 256
            xt_g = xtpool.tile([128, 2, 512], bf16, tag="xt_g", name=f"xt_{ci}")
            nc.sync.dma_start(
                out=xt_g[:, :, 0:w],
                in_=xT[:, ds(ci * 512, w)].rearrange("(cb c) l -> c cb l", c=128),
            )

            # --- qT: two head-pair stacks, bias via ones_row ---
            qT01_ps = ps_q01.tile([128, 512], f32, tag="qT01", name=f"qT01_{ci}")
            qT23_ps = ps_q23.tile([128, 512], f32, tag="qT23", name=f"qT23_{ci}")
            for pair, q_ps in ((0, qT01_ps), (1, qT23_ps)):
                for cb in range(2):
                    nc.tensor.matmul(
                        q_ps[:, 0:w],
                        wq_sb[:, cb, ds(pair * 128, 128)],
                        xt_g[:, cb, 0:w],
                        start=(cb == 0),
                        stop=False,
                    )
                nc.tensor.matmul(
                    q_ps[:, 0:w],
                    bq_sb[:, ds(pair * 128, 128)],
                    ones_row[:, 0:w],
                    start=False,
                    stop=True,
                )
            qc01 = qT_all[:, 0, ci, :]
            qc23 = qT_all[:, 1, ci, :]
            nc.scalar.copy(out=qc01[:, 0:w], in_=qT01_ps[:, 0:w])
            nc.scalar.copy(out=qc23[:, 0:w], in_=qT23_ps[:, 0:w])

            # --- r2 = per-head column sums of q^2 (PE block-ones) ---
            sq01 = spool.tile([128, 512], bf16, tag="sq01", name=f"sq01_{ci}")
            sq23 = spool.tile([128, 512], bf16, tag="sq23", name=f"sq23_{ci}")
            nc.vector.tensor_mul(sq01[:, 0:w], qc01[:, 0:w], qc01[:, 0:w])
            nc.vector.tensor_mul(sq23[:, 0:w], qc23[:, 0:w], qc23[:, 0:w])
            nc.tensor.matmul(
                r2vs[0:4, 0:w], boA, sq01[:, 0:w], start=True, stop=False
            )
            nc.tensor.matmul(
                r2vs[0:4, 0:w], boB, sq23[:, 0:w], start=False, stop=True
            )
            nc.scalar.sqrt(rT_all[:, ci, 0:w], r2vs[0:4, 0:w])

            # --- k,v (normal layout) + kn + kvs, per 256-node group ---
            for gi in range(ngr):
                g = 2 * ci + gi
                kv_ps = ps_kv.tile(
                    [128, 2, 512], f32, tag="kv_ps", name=f"kv_ps_{g}"
                )
                for t in range(2):
                    lhs0 = xt_g[:, 0, ds(gi * 256 + t * 128, 128)]
                    lhs1 = xt_g[:, 1, ds(gi * 256 + t * 128, 128)]
                    nc.tensor.matmul(
                        kv_ps[:, t, :], lhs0, wkv_sb[:, 0, :], start=True, stop=False
                    )
                    nc.tensor.matmul(
                        kv_ps[:, t, :], lhs1, wkv_sb[:, 1, :], start=False, stop=False
                    )
                    nc.tensor.matmul(
                        kv_ps[:, t, :], ones_row[:, 0:128], bkv_sb,
                        start=False, stop=True,
                    )

                # split evacuation: k on ACT, v on DVE; col 512 = ones
                kvbf = spool.tile([128, 2, 513], bf16, tag="kvbf", name=f"kvbf_{g}")
                nc.scalar.copy(out=kvbf[:, :, 0:HD], in_=kv_ps[:, :, 0:HD])
                nc.vector.tensor_copy(kvbf[:, :, ds(HD, HD)], kv_ps[:, :, ds(HD, HD)])
                nc.vector.memset(kvbf[:, :, 512:513], 1.0)
                kbf4 = kvbf[:, :, 0:HD].rearrange("p t (h d) -> p t h d", h=H)
                sqk = spool.tile([128, 2, H, 64], bf16, tag="sqk", name=f"sqk_{g}")
                nc.vector.tensor_mul(sqk, kbf4, kbf4)
                rk2 = spool.tile([128, 2, H, 1], f32, tag="rk2", name=f"rk2_{g}")
                nc.vector.tensor_reduce(
                    rk2, sqk, axis=mybir.AxisListType.X, op=mybir.AluOpType.add
                )
                rk = spool.tile([128, 2, H, 1], f32, tag="rk", name=f"rk_{g}")
                nc.scalar.sqrt(rk, rk2)
                wk_s = spool.tile([128, 2, H, 1], f32, tag="wk_s", name=f"wk_s_{g}")
                nc.vector.reciprocal(wk_s, rk)

                knv = knvpool.tile([128, 2, HD], bf16, tag="knv", name=f"knv_{g}")
                knv4 = knv.rearrange("p t (h d) -> p t h d", h=H)
                nc.vector.tensor_mul(
                    knv4, kbf4, wk_s[:, :, :, :].to_broadcast([128, 2, H, 64])
                )
                if g == NG - 1:
                    nc.vector.tensor_scalar_mul(
                        knv[:, 1, :], knv[:, 1, :], padmask_sb
                    )

                first = g == 0
                last = g == NG - 1
                for t in range(2):
                    st = first and t == 0
                    sp = last and t == 1
                    nc.tensor.matmul(
                        kv01_acc, knv[:, t, 0:128], kvbf[:, t, ds(HD, 257)],
                        start=st, stop=sp,
                    )
                    nc.tensor.matmul(
                        kv23_acc, knv[:, t, ds(128, 128)], kvbf[:, t, ds(HD, 257)],
                        start=st, stop=sp,
                    )
                    nc.tensor.matmul(
                        vs_acc, ones_col, kvbf[:, t, ds(HD, 257)],
                        start=st, stop=sp,
                    )

        # ---- pack + AllReduce ([33,512] f32 = 68KB) -------------------
        # rows 0:32 = kvs blocks (as [64,256]: col-block h = kvs_h),
        # row 32 = [ks 256 | vs 256]
        kv01_sb = post.tile([128, 257], f32, tag="kv01_sb")
        kv23_sb = post.tile([128, 257], f32, tag="kv23_sb")
        vs_sb = post.tile([1, 257], f32, tag="vs_sb")
        nc.vector.tensor_copy(kv01_sb, kv01_acc)
        nc.vector.tensor_copy(kv23_sb, kv23_acc)
        nc.vector.tensor_copy(vs_sb, vs_acc)

        ar_in = dram.tile([33, 512], f32, tag="ar_in")
        ar_out = dram.tile([33, 512], f32, tag="ar_out")
        arv = ar_in[0:32, :].rearrange("p (a c) -> (p a) c", a=2)  # [64, 256]
        blocks = [
            (kv01_sb, 0, 0), (kv01_sb, 64, 64),
            (kv23_sb, 0, 128), (kv23_sb, 64, 192),
        ]
        ar_dmas = []
        for h, (src, r0, c0) in enumerate(blocks):
            ar_dmas.append(
                nc.sync.dma_start(
                    out=arv[:, ds(h * 64, 64)], in_=src[ds(r0, 64), ds(c0, 64)]
                )
            )
            ar_dmas.append(
                nc.scalar.dma_start(
                    out=ar_in[32:33, ds(h * 64, 64)], in_=src[ds(r0, 64), 256:257]
                )
            )
        ar_dmas.append(
            nc.sync.dma_start(out=ar_in[32:33, ds(HD, HD)], in_=vs_sb[:, 0:HD])
        )
        pnop = nc.gpsimd.nop()
        for dd in ar_dmas:
            add_dep_helper(pnop.ins, dd.ins, sync=True, reason="absorb ar_in dma")
        nc.gpsimd.collective_compute(
            "AllReduce",
            mybir.AluOpType.add,
            ins=[ar_in[:, :].opt()],
            outs=[ar_out[:, :].opt()],
            replica_groups=[list(range(NCORES))],
        )

        # ---- post-reduce fixups ---------------------------------------
        # sums row
        sumr = post.tile([1, 512], f32, tag="sumr")
        nc.sync.dma_start(out=sumr, in_=ar_out[32:33, :])
        arov = ar_out[0:32, :].rearrange("p (a c) -> (p a) c", a=2)  # [64, 256]

        # stacked kvs lhsT tiles [128, 64]: rows 0:64 = kvs_h(even),
        # 64:128 = kvs_h(odd); then += ks (x) bv and cast to bf16
        kvstk = []
        for j in range(2):  # pair 01, 23
            stk = post.tile([128, D], f32, tag=f"kvstk{j}")
            nc.sync.dma_start(out=stk[0:64, :], in_=arov[:, ds((2 * j) * 64, 64)])
            nc.sync.dma_start(
                out=stk[64:128, :], in_=arov[:, ds((2 * j + 1) * 64, 64)]
            )
            kvstk.append(stk)
        # ks columns stacked to match: [128,1] per pair
        ksc = post.tile([128, 2, 1], f32, tag="ksc")
        for j in range(2):
            for i in range(2):
                h = 2 * j + i
                nc.sync.dma_start(
                    out=ksc[ds(i * 64, 64), j, :],
                    in_=ar_out[32:33, ds(h * 64, 64)],
                )
        tmpo = post.tile([128, D], f32, tag="tmpo")
        lhs_bf = []
        for j, bvbc in ((0, bv01_bc), (1, bv23_bc)):
            nc.vector.tensor_scalar_mul(tmpo, bvbc, ksc[:, j, :])
            lb = post.tile([128, D], bf16, tag=f"lhsbf{j}")
            nc.vector.tensor_add(lb, kvstk[j], tmpo)
            lhs_bf.append(lb)
        # vs rows [4, 64] + N*bv, bf16
        vs4 = post.tile([4, D], f32, tag="vs4")
        nc.sync.dma_start(out=vs4, in_=ar_out[32:33, ds(HD, HD)])
        vsM = post.tile([4, D], bf16, tag="vsM")
        nc.vector.tensor_scalar(
            tmpo[0:4, :], bv4, float(N_FULL), None, op0=mybir.AluOpType.mult
        )
        nc.vector.tensor_add(vsM, vs4, tmpo[0:4, :])
        # den lhsT tiles: ksM01/ksM23 [128, 4] = 4*ks placed block-diagonally
        ksM = post.tile([128, 2, 4], bf16, tag="ksM")
        ks4 = post.tile([128, 2, 1], f32, tag="ks4")
        nc.vector.tensor_scalar(
            ks4, ksc, 4.0, None, op0=mybir.AluOpType.mult
        )
        nc.vector.memset(ksM, 0.0)
        for j in range(2):
            for i in range(2):
                nc.vector.tensor_copy(
                    ksM[ds(i * 64, 64), j, 2 * j + i : 2 * j + i + 1],
                    ks4[ds(i * 64, 64), j, :],
                )
        # Ndiag [4,4] = 4N * I
        ndiag = post.tile([4, 4], bf16, tag="ndiag")
        nc.vector.tensor_scalar(
            ndiag, ident[0:4, 0:4], 4.0 * N_FULL, None, op0=mybir.AluOpType.mult
        )

        # ---- phase D: den -> rec -> bc -> scale -> num -> store -------
        for ci in range(NCH):
            w = chunk_width(ci)
            qc01 = qT_all[:, 0, ci, 0:w]
            qc23 = qT_all[:, 1, ci, 0:w]
            rT = rT_all[:, ci, 0:w]

            den_ps = r2vs[0:4, :]
            nc.tensor.matmul(den_ps[:, 0:w], ksM[:, 0, :], qc01, start=True, stop=False)
            nc.tensor.matmul(den_ps[:, 0:w], ksM[:, 1, :], qc23, start=False, stop=False)
            nc.tensor.matmul(den_ps[:, 0:w], ndiag, rT, start=False, stop=True)
            rec = dpool.tile([4, 512], bf16, tag="rec", name=f"rec_{ci}")
            with nc.allow_low_precision(reason="bf16 1/den: 0.4% rel, budget 2e-2"):
                nc.vector.reciprocal(rec[:, 0:w], den_ps[:, 0:w])

            bc01_ps = ps_q01.tile([128, 512], f32, tag="qT01", name=f"bc01_{ci}")
            bc23_ps = ps_q23.tile([128, 512], f32, tag="qT23", name=f"bc23_{ci}")
            nc.tensor.matmul(bc01_ps[:, 0:w], sel01, rec[:, 0:w], start=True, stop=True)
            nc.tensor.matmul(bc23_ps[:, 0:w], sel23, rec[:, 0:w], start=True, stop=True)

            qs01 = dpool.tile([128, 512], bf16, tag="qs01", name=f"qs01_{ci}")
            qs23 = dpool.tile([128, 512], bf16, tag="qs23", name=f"qs23_{ci}")
            nc.vector.tensor_mul(qs01[:, 0:w], qc01, bc01_ps[:, 0:w])
            nc.vector.tensor_mul(qs23[:, 0:w], qc23, bc23_ps[:, 0:w])
            rs = dpool.tile([4, 512], bf16, tag="rs", name=f"rs_{ci}")
            nc.vector.tensor_mul(rs[:, 0:w], rT, rec[:, 0:w])

            o_ps = ps_kv.tile([64, 512], f32, tag="kv_ps", name=f"o_ps_{ci}")
            nc.tensor.matmul(o_ps[:, 0:w], lhs_bf[0], qs01[:, 0:w], start=True, stop=False)
            nc.tensor.matmul(o_ps[:, 0:w], lhs_bf[1], qs23[:, 0:w], start=False, stop=False)
            nc.tensor.matmul(o_ps[:, 0:w], vsM, rs[:, 0:w], start=False, stop=True)
            osb = opool.tile([64, 512], f32, tag="osb", name=f"osb_{ci}")
            nc.scalar.copy(out=osb[:, 0:w], in_=o_ps[:, 0:w])
            eng = nc.gpsimd if ci % 2 == 0 else nc.sync
            eng.dma_start(out=outT[:, ds(ci * 512, w)], in_=osb[:, 0:w])

    nc.finalize()
    return nc


def _get_nc():
    global _nc_cache
    if _nc_cache is None:
        _nc_cache = _build_nc()
    return _nc_cache


def kernel(x, Wq_w, Wq_b, Wk_w, Wk_b, Wv_w, Wv_b, n_nodes=None):
    import ml_dtypes
    from concourse.bass_utils import run_bass_kernel_spmd

    x = np.asarray(x, np.float32)
    xp = np.zeros((NCORES, NL, IN), np.float32)
    xp[:, :NLOC, :] = x.reshape(NCORES, NLOC, IN)

    wqT = np.ascontiguousarray(np.asarray(Wq_w, np.float32).T)
    wkT = np.ascontiguousarray(np.asarray(Wk_w, np.float32).T)
    wvT = np.ascontiguousarray(np.asarray(Wv_w, np.float32).T)
    bq = np.asarray(Wq_b, np.float32).reshape(1, HD)
    bk = np.asarray(Wk_b, np.float32).reshape(1, HD)
    bv = np.asarray(Wv_b, np.float32).reshape(1, HD)
    pm = np.ones((128, 1), np.float32)
    pm[PAD0:, 0] = 0.0
    import ml_dtypes as _md

    selc = np.zeros((8, 128), _md.bfloat16)
    selc[0, 0:64] = 1
    selc[1, 64:128] = 1
    selc[6, 0:64] = 1
    selc[7, 64:128] = 1

    nc = _get_nc()
    in_maps = []
    for c in range(NCORES):
        xTb = np.ascontiguousarray(xp[c].T).astype(ml_dtypes.bfloat16)
        in_maps.append(
            {
                "xT": xTb,
                "wqT": wqT,
                "wkT": wkT,
                "wvT": wvT,
                "bq": bq,
                "bk": bk,
                "bv": bv,
                "padmask": pm,
                "selc": selc,
            }
        )
    import os

    trace = bool(os.environ.get("KERNEL_TRACE"))
    stitch = bool(os.environ.get("KERNEL_TRACE_STITCH"))
    tcores = os.environ.get("KERNEL_TRACE_CORES", "0")
    trace_cores = [int(c) for c in tcores.split(",")] if trace else None
    res = run_bass_kernel_spmd(
        nc,
        in_maps,
        core_ids=list(range(NCORES)),
        trace=trace,
        trace_cores=trace_cores,
        stitch_traces=stitch,
    )
    global _last_result
    _last_result = res
    outs = [
        np.ascontiguousarray(res.results[c]["outT"].T[:NLOC, :])
        for c in range(NCORES)
    ]
    return np.concatenate(outs, axis=0).astype(np.float32)


if __name__ == "__main__":
    rng = np.random.default_rng(0)
    s = 1.0 / np.sqrt(IN)
    inputs = {
        "x": rng.standard_normal((N_FULL, IN)).astype(np.float32),
        "Wq_w": rng.uniform(-s, s, (HD, IN)).astype(np.float32),
        "Wq_b": rng.uniform(-s, s, HD).astype(np.float32),
        "Wk_w": rng.uniform(-s, s, (HD, IN)).astype(np.float32),
        "Wk_b": rng.uniform(-s, s, HD).astype(np.float32),
        "Wv_w": rng.uniform(-s, s, (HD, IN)).astype(np.float32),
        "Wv_b": rng.uniform(-s, s, HD).astype(np.float32),
        "n_nodes": np.array([N_FULL], np.int32),
    }
    o = kernel(**inputs)
    print(o.shape, o.dtype, np.abs(o).max())
